# revision 1
# baseline (speedup 1.0000x reference)
import numpy as np
import jax
import jax.numpy as jnp
from functools import partial
from jax.sharding import Mesh, PartitionSpec as P
from jax.experimental.shard_map import shard_map

GROUPS = 8
KS = 64          # kernel_size == H
IN_PLANES = 128
OUT_PLANES = 128
GP = OUT_PLANES // GROUPS  # 16
EPS = 1e-5
N_CORES = 8
AXIS = "cores"


def _bn_dist(x, gamma, beta, axes):
    # training-mode batchnorm, channel axis 1; stats are GLOBAL across the
    # batch axis which is sharded over cores -> pmean to get exact stats.
    m = jax.lax.pmean(jnp.mean(x, axes, keepdims=True), AXIS)
    m2 = jax.lax.pmean(jnp.mean(x * x, axes, keepdims=True), AXIS)
    v = m2 - m * m
    shape = [1] * x.ndim
    shape[1] = -1
    return (x - m) * jax.lax.rsqrt(v + EPS) * gamma.reshape(shape) + beta.reshape(shape)


def _body(x, w_qkv, g_qkv, b_qkv, g_sim, b_sim, g_out, b_out, w1, b1, w2, b2, relative):
    # x: [N_shard, C, H, W] (sharded over N)
    N, C, H, W = x.shape
    x = jnp.transpose(x, (0, 3, 1, 2)).reshape(N * W, C, H)

    # squeeze-excite gating branch (per-row, no cross-core deps)
    xn = x.mean(axis=2)
    xn = jax.nn.relu(xn @ w1.T + b1)
    xn = jax.nn.relu(xn @ w2.T + b2)
    sig = jax.nn.sigmoid(xn)
    sig1 = sig[:, 0, None, None, None]
    sig2 = sig[:, 1, None, None, None]
    sig3 = sig[:, 2, None, None, None]
    sig4 = sig[:, 3, None, None, None]

    qkv = jnp.einsum('oc,bch->boh', w_qkv, x)
    qkv = _bn_dist(qkv, g_qkv, b_qkv, (0, 2))
    qkv = qkv.reshape(N * W, GROUPS, GP * 2, H)
    q = qkv[:, :, :GP // 2]
    k = qkv[:, :, GP // 2:GP]
    v = qkv[:, :, GP:]

    rel_idx = jnp.arange(KS)[:, None] - jnp.arange(KS)[None, :] + KS - 1
    all_emb = relative[:, rel_idx]
    q_emb = all_emb[:GP // 2]
    k_emb = all_emb[GP // 2:GP]
    v_emb = all_emb[GP:]

    qr = sig1 * jnp.einsum('bgci,cij->bgij', q, q_emb)
    kr = sig2 * jnp.einsum('bgci,cij->bgji', k, k_emb)
    qk = jnp.einsum('bgci,bgcj->bgij', q, k)

    stacked = jnp.concatenate([qk, qr, kr], axis=1)
    stacked = _bn_dist(stacked, g_sim, b_sim, (0, 2, 3))
    sim = stacked.reshape(N * W, 3, GROUPS, H, H).sum(axis=1)
    sim = jax.nn.softmax(sim, axis=3)

    sv = sig3 * jnp.einsum('bgij,bgcj->bgci', sim, v)
    sve = sig4 * jnp.einsum('bgij,cij->bgci', sim, v_emb)
    out = jnp.concatenate([sv, sve], axis=-1).reshape(N * W, OUT_PLANES * 2, H)
    out = _bn_dist(out, g_out, b_out, (0, 2))
    out = out.reshape(N, W, OUT_PLANES, 2, H).sum(axis=3)
    return jnp.transpose(out, (0, 2, 3, 1))


_jitted = None


def _get_fn():
    global _jitted
    if _jitted is None:
        devices = jax.devices()[:N_CORES]
        mesh = Mesh(np.asarray(devices), (AXIS,))
        reps = (P(),) * 12
        fn = shard_map(
            _body, mesh=mesh,
            in_specs=(P(AXIS),) + reps,
            out_specs=P(AXIS),
            check_rep=False,
        )
        _jitted = jax.jit(fn)
    return _jitted


def kernel(x, w_qkv, g_qkv, b_qkv, g_sim, b_sim, g_out, b_out, w1, b1, w2, b2, relative):
    fn = _get_fn()
    out = fn(jnp.asarray(x), jnp.asarray(w_qkv), jnp.asarray(g_qkv),
             jnp.asarray(b_qkv), jnp.asarray(g_sim), jnp.asarray(b_sim),
             jnp.asarray(g_out), jnp.asarray(b_out), jnp.asarray(w1),
             jnp.asarray(b1), jnp.asarray(w2), jnp.asarray(b2),
             jnp.asarray(relative))
    return np.asarray(jax.device_get(out)).astype(np.float32)
